# revision 4
# baseline (speedup 1.0000x reference)
"""Sliding-window causal GQA attention with ALiBi, head-sharded across 8 TRN2 cores.

Full problem: B=2, S=2048, H=32, D=128, KV=8 (GQA group 4), window=(1024,0),
softmax scale 1/sqrt(128), ALiBi slopes = 0.8409^(h+1).
Sharding: core c owns heads 4c..4c+3 and KV head c. No collectives.
"""

import math
import sys
from contextlib import ExitStack

import numpy as np

sys.path.insert(0, "/opt/trn_rl_repo")

import concourse.bass as bass
import concourse.mybir as mybir
import concourse.tile as tile
from concourse import bacc
from concourse.bass_utils import run_bass_kernel_spmd
from concourse.masks import make_identity

B, S = 2, 2048
H, D = 32, 128
KV = 8
WINDOW = 1024
SCALE = 1.0 / math.sqrt(D)
HPC = H // 8          # heads per core
NQ = S // 128         # 16 query blocks per batch
NDELTA = 9            # kj in [qi-8, qi]
NEG = -1e30

F32 = mybir.dt.float32
BF16 = mybir.dt.bfloat16


def _slopes():
    start = 2.0 ** (-(2.0 ** (-(math.log2(H) - 3))))
    return [start * start**i for i in range(H)]


def build_kernel(core_heads_slopes):
    """One SPMD graph; per-core differences live only in the input data, so the
    alibi tables (which differ by head) must be identical across cores — they are
    built from slopes of heads 4c..4c+3, which differ per core. SPMD requires one
    graph, so instead alibi tables for all 32 heads are built on every core and
    the per-core head offset is passed as a tiny input tensor?  Simpler: build
    8 separate graphs is not allowed (one NEFF).  Solution used here: the alibi
    table is itself an *input* (computed on host, DMA'd in), so the single graph
    stays head-agnostic."""
    nc = bacc.Bacc("TRN2", target_bir_lowering=False, debug=False)

    q_d = nc.dram_tensor("q", [B * S, HPC * D], F32, kind="ExternalInput").ap()
    k_d = nc.dram_tensor("k", [B * S, D], F32, kind="ExternalInput").ap()
    v_d = nc.dram_tensor("v", [B * S, D], F32, kind="ExternalInput").ap()
    # alibi tables: per head (4), columns ordered delta=8..0, each 128 wide, f32
    a_d = nc.dram_tensor("alibi", [128, HPC * NDELTA * 128], F32, kind="ExternalInput").ap()
    o_d = nc.dram_tensor("out", [B * S, HPC * D], F32, kind="ExternalOutput").ap()

    with tile.TileContext(nc) as tc, ExitStack() as ctx:
        const = ctx.enter_context(tc.tile_pool(name="const", bufs=1))
        kvp = ctx.enter_context(tc.tile_pool(name="kv", bufs=1))
        ldp = ctx.enter_context(tc.tile_pool(name="ld", bufs=3))
        qp = ctx.enter_context(tc.tile_pool(name="qp", bufs=3))
        pp = ctx.enter_context(tc.tile_pool(name="pp", bufs=4))
        outp = ctx.enter_context(tc.tile_pool(name="outp", bufs=3))
        dnp = ctx.enter_context(tc.tile_pool(name="dnp", bufs=3))
        ps_s = ctx.enter_context(tc.tile_pool(name="ps_s", bufs=2, space="PSUM"))
        ps_t = ctx.enter_context(tc.tile_pool(name="ps_t", bufs=2, space="PSUM"))
        ps_o = ctx.enter_context(tc.tile_pool(name="ps_o", bufs=2, space="PSUM"))

        ident = const.tile([128, 128], BF16)
        make_identity(nc, ident[:])

        # alibi tables, resident
        atab = const.tile([128, HPC * NDELTA * 128], F32)
        nc.sync.dma_start(atab[:], a_d[:, :])

        # ---- preload K^T and V (bf16) for both batches ----
        kt = kvp.tile([128, B * S], BF16)   # [d, token] per batch contiguous
        vt = kvp.tile([128, B * S], BF16)   # [token%128, (b,kj)*128 + d]
        k_r = k_d.rearrange("(n p) d -> n p d", p=128)
        v_r = v_d.rearrange("(n p) d -> n p d", p=128)
        for t in range(B * S // 128):
            kf = ldp.tile([128, 128], F32, tag="kf")
            nc.sync.dma_start(kf[:], k_r[t, :, :])
            kb = ldp.tile([128, 128], BF16, tag="kb")
            nc.vector.tensor_copy(kb[:], kf[:])
            ktp = ps_t.tile([128, 128], BF16, tag="tps")
            nc.tensor.transpose(ktp[:], kb[:], ident[:])
            nc.scalar.copy(kt[:, t * 128 : (t + 1) * 128], ktp[:])

            vf = ldp.tile([128, 128], F32, tag="kf")
            nc.sync.dma_start(vf[:], v_r[t, :, :])
            nc.vector.tensor_copy(vt[:, t * 128 : (t + 1) * 128], vf[:])

        q_r = q_d.rearrange("(n p) hd -> n p hd", p=128)
        o_r = o_d.rearrange("(n p) hd -> n p hd", p=128)

        for h in range(HPC):
            for b in range(B):
                for qi in range(NQ):
                    tok = b * NQ + qi
                    # load Q tile, cast, transpose -> QT [d, q]
                    qf = qp.tile([128, 128], F32, tag="qf")
                    nc.sync.dma_start(qf[:], q_r[tok, :, h * D : (h + 1) * D])
                    qb = qp.tile([128, 128], BF16, tag="qb")
                    nc.vector.tensor_copy(qb[:], qf[:])
                    qtp = ps_t.tile([128, 128], BF16, tag="tps")
                    nc.tensor.transpose(qtp[:], qb[:], ident[:])
                    qtb = qp.tile([128, 128], BF16, tag="qtb")
                    nc.scalar.copy(qtb[:], qtp[:])

                    kj0 = max(0, qi - 8)
                    nkj = qi - kj0 + 1
                    o_ps = ps_o.tile([128, 128], F32, tag="ops")
                    den = dnp.tile([128, 3], F32, tag="den")
                    nchunk = (nkj + 3) // 4
                    for ci in range(nchunk):
                        c0 = kj0 + ci * 4          # first kj of chunk
                        w = min(4, kj0 + nkj - c0)  # kj blocks in chunk
                        wc = w * 128
                        s_ps = ps_s.tile([128, 512], F32, tag="sps")
                        nc.tensor.matmul(
                            s_ps[:, :wc],
                            qtb[:],
                            kt[:, (b * S + c0 * 128) : (b * S + c0 * 128 + wc)],
                        )
                        # alibi slice: delta of first kj in chunk = qi - c0
                        d_hi = qi - c0
                        acol = h * NDELTA * 128 + (8 - d_hi) * 128
                        t_sb = pp.tile([128, 512], F32, tag="tsb")
                        nc.vector.scalar_tensor_tensor(
                            t_sb[:, :wc],
                            s_ps[:, :wc],
                            SCALE,
                            atab[:, acol : acol + wc],
                            op0=mybir.AluOpType.mult,
                            op1=mybir.AluOpType.add,
                        )
                        p_sb = pp.tile([128, 512], BF16, tag="psb")
                        nc.scalar.activation(
                            p_sb[:, :wc],
                            t_sb[:, :wc],
                            mybir.ActivationFunctionType.Exp,
                            accum_out=den[:, ci : ci + 1],
                        )
                        for j in range(w):
                            kj = c0 + j
                            pt_ps = ps_t.tile([128, 128], BF16, tag="tps")
                            nc.tensor.transpose(
                                pt_ps[:], p_sb[:, j * 128 : (j + 1) * 128], ident[:]
                            )
                            pt_sb = pp.tile([128, 128], BF16, tag="ptsb")
                            nc.scalar.copy(pt_sb[:], pt_ps[:])
                            nc.tensor.matmul(
                                o_ps[:],
                                pt_sb[:],
                                vt[:, (b * S + kj * 128) : (b * S + (kj + 1) * 128)],
                                start=(kj == kj0),
                                stop=(kj == qi),
                            )
                    dsum = dnp.tile([128, 1], F32, tag="dsum")
                    if nchunk > 1:
                        nc.vector.reduce_sum(
                            dsum[:], den[:, :nchunk], axis=mybir.AxisListType.X
                        )
                    else:
                        nc.vector.tensor_copy(dsum[:], den[:, :1])
                    drec = dnp.tile([128, 1], F32, tag="drec")
                    nc.vector.reciprocal(drec[:], dsum[:])
                    o_sb = outp.tile([128, 128], F32, tag="osb")
                    nc.vector.tensor_scalar_mul(o_sb[:], o_ps[:], drec[:])
                    nc.sync.dma_start(o_r[tok, :, h * D : (h + 1) * D], o_sb[:])
    nc.compile()
    return nc


_NC = None


def _get_nc():
    global _NC
    if _NC is None:
        _NC = build_kernel(None)
    return _NC


def _alibi_tables(slopes):
    """[128, HPC*9*128] f32: per head, columns delta=8..0; entry = -slope*(128d + r - c),
    masked to NEG where invalid (causal on d=0, window edge on d=8)."""
    r = np.arange(128)[:, None]
    c = np.arange(128)[None, :]
    cols = []
    for s in slopes:
        for d in range(8, -1, -1):
            a = -s * (128 * d + r - c)
            if d == 0:
                a = np.where(c > r, NEG, a)
            if d == 8:
                a = np.where(c < r, NEG, a)
            cols.append(a)
    return np.concatenate(cols, axis=1).astype(np.float32)


def kernel(q, k, v):
    nc = _get_nc()
    slopes = _slopes()
    in_maps = []
    for core in range(8):
        in_maps.append(
            {
                "q": np.ascontiguousarray(q[:, core * HPC * D : (core + 1) * HPC * D]),
                "k": np.ascontiguousarray(k[:, core * D : (core + 1) * D]),
                "v": np.ascontiguousarray(v[:, core * D : (core + 1) * D]),
                "alibi": _alibi_tables(slopes[core * HPC : (core + 1) * HPC]),
            }
        )
    res = run_bass_kernel_spmd(nc, in_maps, core_ids=list(range(8)))
    return np.concatenate([res.results[i]["out"] for i in range(8)], axis=1)


# revision 5
# speedup vs baseline: 1.1253x; 1.1253x over previous
"""Sliding-window causal GQA attention with ALiBi, head-sharded across 8 TRN2 cores.

Full problem: B=2, S=2048, H=32, D=128, KV=8 (GQA group 4), window=(1024,0),
softmax scale 1/sqrt(128), ALiBi slopes = 0.8409^(h+1).
Sharding: core c owns heads 4c..4c+3 and KV head c. No collectives.
"""

import math
import sys
from contextlib import ExitStack

import numpy as np

sys.path.insert(0, "/opt/trn_rl_repo")

import concourse.bass as bass
import concourse.mybir as mybir
import concourse.tile as tile
from concourse import bacc
from concourse.bass_utils import run_bass_kernel_spmd
from concourse.masks import make_identity

B, S = 2, 2048
H, D = 32, 128
KV = 8
WINDOW = 1024
SCALE = 1.0 / math.sqrt(D)
HPC = H // 8          # heads per core
NQ = S // 128         # 16 query blocks per batch
NDELTA = 9            # kj in [qi-8, qi]
NEG = -1e30

F32 = mybir.dt.float32
BF16 = mybir.dt.bfloat16


def _slopes():
    start = 2.0 ** (-(2.0 ** (-(math.log2(H) - 3))))
    return [start * start**i for i in range(H)]


def build_kernel(core_heads_slopes):
    """One SPMD graph; per-core differences live only in the input data, so the
    alibi tables (which differ by head) must be identical across cores — they are
    built from slopes of heads 4c..4c+3, which differ per core. SPMD requires one
    graph, so instead alibi tables for all 32 heads are built on every core and
    the per-core head offset is passed as a tiny input tensor?  Simpler: build
    8 separate graphs is not allowed (one NEFF).  Solution used here: the alibi
    table is itself an *input* (computed on host, DMA'd in), so the single graph
    stays head-agnostic."""
    nc = bacc.Bacc("TRN2", target_bir_lowering=False, debug=False)

    q_d = nc.dram_tensor("q", [B * S, HPC * D], F32, kind="ExternalInput").ap()
    k_d = nc.dram_tensor("k", [B * S, D], F32, kind="ExternalInput").ap()
    v_d = nc.dram_tensor("v", [B * S, D], F32, kind="ExternalInput").ap()
    # alibi tables: per head (4), columns ordered delta=8..0, each 128 wide, f32
    a_d = nc.dram_tensor("alibi", [128, HPC * NDELTA * 128], F32, kind="ExternalInput").ap()
    o_d = nc.dram_tensor("out", [B * S, HPC * D], F32, kind="ExternalOutput").ap()

    with tile.TileContext(nc) as tc, ExitStack() as ctx:
        const = ctx.enter_context(tc.tile_pool(name="const", bufs=1))
        kvp = ctx.enter_context(tc.tile_pool(name="kv", bufs=1))
        ldp = ctx.enter_context(tc.tile_pool(name="ld", bufs=3))
        qp = ctx.enter_context(tc.tile_pool(name="qp", bufs=3))
        pp = ctx.enter_context(tc.tile_pool(name="pp", bufs=4))
        outp = ctx.enter_context(tc.tile_pool(name="outp", bufs=3))
        dnp = ctx.enter_context(tc.tile_pool(name="dnp", bufs=3))
        ps_s = ctx.enter_context(tc.tile_pool(name="ps_s", bufs=2, space="PSUM"))
        ps_t = ctx.enter_context(tc.tile_pool(name="ps_t", bufs=2, space="PSUM"))
        ps_o = ctx.enter_context(tc.tile_pool(name="ps_o", bufs=2, space="PSUM"))

        ident = const.tile([128, 128], BF16)
        make_identity(nc, ident[:])

        # alibi tables, resident
        atab = const.tile([128, HPC * NDELTA * 128], F32)
        nc.sync.dma_start(atab[:], a_d[:, :])

        # ---- preload K^T and V (bf16) for both batches ----
        kt = kvp.tile([128, B * S], BF16)   # [d, token] per batch contiguous
        vt = kvp.tile([128, B * S], BF16)   # [token%128, (b,kj)*128 + d]
        k_r = k_d.rearrange("(n p) d -> n p d", p=128)
        v_r = v_d.rearrange("(n p) d -> n p d", p=128)
        for t in range(B * S // 128):
            kf = ldp.tile([128, 128], F32, tag="kf")
            nc.sync.dma_start(kf[:], k_r[t, :, :])
            kb = ldp.tile([128, 128], BF16, tag="kb")
            nc.vector.tensor_copy(kb[:], kf[:])
            ktp = ps_t.tile([128, 128], BF16, tag="tps")
            nc.tensor.transpose(ktp[:], kb[:], ident[:])
            nc.scalar.copy(kt[:, t * 128 : (t + 1) * 128], ktp[:])

            vf = ldp.tile([128, 128], F32, tag="kf")
            nc.sync.dma_start(vf[:], v_r[t, :, :])
            nc.vector.tensor_copy(vt[:, t * 128 : (t + 1) * 128], vf[:])

        q_r = q_d.rearrange("(n p) hd -> n p hd", p=128)
        o_r = o_d.rearrange("(n p) hd -> n p hd", p=128)

        for h in range(HPC):
            for b in range(B):
                for qi in range(NQ):
                    tok = b * NQ + qi
                    # load Q tile, cast, transpose -> QT [d, q]
                    qf = qp.tile([128, 128], F32, tag="qf")
                    nc.sync.dma_start(qf[:], q_r[tok, :, h * D : (h + 1) * D])
                    qb = qp.tile([128, 128], BF16, tag="qb")
                    nc.vector.tensor_copy(qb[:], qf[:])
                    qtp = ps_t.tile([128, 128], BF16, tag="tps")
                    nc.tensor.transpose(qtp[:], qb[:], ident[:])
                    qtb = qp.tile([128, 128], BF16, tag="qtb")
                    nc.scalar.copy(qtb[:], qtp[:])

                    kj0 = max(0, qi - 8)
                    nkj = qi - kj0 + 1
                    o_ps = ps_o.tile([128, 128], F32, tag="ops")
                    den = dnp.tile([128, 3], F32, tag="den")
                    nchunk = (nkj + 3) // 4
                    for ci in range(nchunk):
                        c0 = kj0 + ci * 4          # first kj of chunk
                        w = min(4, kj0 + nkj - c0)  # kj blocks in chunk
                        wc = w * 128
                        s_ps = ps_s.tile([128, 512], F32, tag="sps")
                        nc.tensor.matmul(
                            s_ps[:, :wc],
                            qtb[:],
                            kt[:, (b * S + c0 * 128) : (b * S + c0 * 128 + wc)],
                        )
                        # alibi slice: delta of first kj in chunk = qi - c0
                        d_hi = qi - c0
                        acol = h * NDELTA * 128 + (8 - d_hi) * 128
                        t_sb = pp.tile([128, 512], F32, tag="tsb")
                        nc.vector.scalar_tensor_tensor(
                            t_sb[:, :wc],
                            s_ps[:, :wc],
                            SCALE,
                            atab[:, acol : acol + wc],
                            op0=mybir.AluOpType.mult,
                            op1=mybir.AluOpType.add,
                        )
                        p_sb = pp.tile([128, 512], BF16, tag="psb")
                        nc.scalar.activation(
                            p_sb[:, :wc],
                            t_sb[:, :wc],
                            mybir.ActivationFunctionType.Exp,
                            accum_out=den[:, ci : ci + 1],
                        )
                        for j in range(w):
                            kj = c0 + j
                            pt_ps = ps_t.tile([128, 128], BF16, tag="tps")
                            nc.tensor.transpose(
                                pt_ps[:], p_sb[:, j * 128 : (j + 1) * 128], ident[:]
                            )
                            pt_sb = pp.tile([128, 128], BF16, tag="ptsb")
                            if j % 2 == 0:
                                nc.scalar.copy(pt_sb[:], pt_ps[:])
                            else:
                                nc.vector.tensor_copy(pt_sb[:], pt_ps[:])
                            nc.tensor.matmul(
                                o_ps[:],
                                pt_sb[:],
                                vt[:, (b * S + kj * 128) : (b * S + (kj + 1) * 128)],
                                start=(kj == kj0),
                                stop=(kj == qi),
                            )
                    dsum = dnp.tile([128, 1], F32, tag="dsum")
                    if nchunk > 1:
                        nc.vector.reduce_sum(
                            dsum[:], den[:, :nchunk], axis=mybir.AxisListType.X
                        )
                    else:
                        nc.vector.tensor_copy(dsum[:], den[:, :1])
                    drec = dnp.tile([128, 1], F32, tag="drec")
                    nc.vector.reciprocal(drec[:], dsum[:])
                    o_sb = outp.tile([128, 128], F32, tag="osb")
                    nc.vector.tensor_scalar_mul(o_sb[:], o_ps[:], drec[:])
                    nc.sync.dma_start(o_r[tok, :, h * D : (h + 1) * D], o_sb[:])
    nc.compile()
    return nc


_NC = None


def _get_nc():
    global _NC
    if _NC is None:
        _NC = build_kernel(None)
    return _NC


def _alibi_tables(slopes):
    """[128, HPC*9*128] f32: per head, columns delta=8..0; entry = -slope*(128d + r - c),
    masked to NEG where invalid (causal on d=0, window edge on d=8)."""
    r = np.arange(128)[:, None]
    c = np.arange(128)[None, :]
    cols = []
    for s in slopes:
        for d in range(8, -1, -1):
            a = -s * (128 * d + r - c)
            if d == 0:
                a = np.where(c > r, NEG, a)
            if d == 8:
                a = np.where(c < r, NEG, a)
            cols.append(a)
    return np.concatenate(cols, axis=1).astype(np.float32)


def kernel(q, k, v):
    nc = _get_nc()
    slopes = _slopes()
    in_maps = []
    for core in range(8):
        in_maps.append(
            {
                "q": np.ascontiguousarray(q[:, core * HPC * D : (core + 1) * HPC * D]),
                "k": np.ascontiguousarray(k[:, core * D : (core + 1) * D]),
                "v": np.ascontiguousarray(v[:, core * D : (core + 1) * D]),
                "alibi": _alibi_tables(slopes[core * HPC : (core + 1) * HPC]),
            }
        )
    res = run_bass_kernel_spmd(nc, in_maps, core_ids=list(range(8)))
    return np.concatenate([res.results[i]["out"] for i in range(8)], axis=1)
